# revision 1
# baseline (speedup 1.0000x reference)
"""Trainium2 Bass kernel for nn_LinearWithGroupedConv (out = x @ weight.T).

Full-input contract: kernel(x=[4,2048,4096] f32, weight=[4096,4096] f32)
-> [4,2048,4096] f32.

Strategy (tensor-parallel, column sharding per the hint):
  - out[s, o] = sum_k x[s, k] * weight[o, k];  S=8192 (4*2048), K=4096, O=4096.
  - Shard `weight` over out_feature across 8 cores (512 columns each),
    replicate x. Each core computes out_shard [8192, 512]; host concats.
  - On host: transpose x -> xT [K, S] and weight -> wT [K, O] so the
    contraction dim lands on SBUF partitions, and cast to fp16 (PSUM
    accumulation is fp32; fp16 keeps 10 mantissa bits -> rel err ~2e-4).
  - Per core: keep the full wT shard resident in SBUF ([128, 32, 512] fp16),
    stream xT in 4 MB chunks ([128, 32, 512] fp16), accumulate 32 matmuls
    (K-tiles) per 128-row output tile into one PSUM bank, copy to SBUF via
    DVE, DMA out.
"""

import ml_dtypes
import numpy as np

import concourse.bass as bass
import concourse.mybir as mybir
import concourse.tile as tile
from concourse import bacc
from concourse.bass_utils import run_bass_kernel_spmd

N_CORES = 8
S = 8192          # 4 * 2048 sequence rows
K = 4096          # in_feature (contraction)
O = 4096          # out_feature
O_SHARD = O // N_CORES          # 512
P = 128
K_TILES = K // P                # 32
S_CHUNK = 512                   # seq columns per streamed x chunk
S_SUB = S_CHUNK // P            # 4 psum tiles per chunk
N_CHUNKS = S // S_CHUNK         # 16

# "fp16": single-pass fp16 matmul (rel err ~2e-4)
# "split": 3-pass fp16 hi/lo split (rel err ~1e-5, 3x compute)
MODE = "fp16"
PROFILE = False          # test.py sets True to capture an NTFF trace
LAST_PROFILE = None      # BassKernelResults of the last run when PROFILE

_CACHE = {}


def _build_fp16(split: bool, dt16=mybir.dt.float16):
    nc = bacc.Bacc(None, target_bir_lowering=False)
    n_terms = 3 if split else 1

    xs = []
    ws = []
    if split:
        names = [("x_hi", "w_hi"), ("x_hi", "w_lo"), ("x_lo", "w_hi")]
        x_hi = nc.dram_tensor("x_hi", [K, S], dt16, kind="ExternalInput")
        x_lo = nc.dram_tensor("x_lo", [K, S], dt16, kind="ExternalInput")
        w_hi = nc.dram_tensor("w_hi", [K, O_SHARD], dt16, kind="ExternalInput")
        w_lo = nc.dram_tensor("w_lo", [K, O_SHARD], dt16, kind="ExternalInput")
        handles = {"x_hi": x_hi, "x_lo": x_lo, "w_hi": w_hi, "w_lo": w_lo}
        x_names = ["x_hi", "x_lo"]
        w_names = ["w_hi", "w_lo"]
    else:
        names = [("x", "w")]
        handles = {
            "x": nc.dram_tensor("x", [K, S], dt16, kind="ExternalInput"),
            "w": nc.dram_tensor("w", [K, O_SHARD], dt16, kind="ExternalInput"),
        }
        x_names = ["x"]
        w_names = ["w"]

    out = nc.dram_tensor("out", [S, O_SHARD], mybir.dt.float32, kind="ExternalOutput")

    with tile.TileContext(nc) as tc:
        with (
            tc.tile_pool(name="wpool", bufs=1) as wpool,
            tc.tile_pool(name="xpool", bufs=2) as xpool,
            tc.tile_pool(name="x0pool", bufs=1) as x0pool,
            tc.tile_pool(name="opool", bufs=4) as opool,
            tc.tile_pool(name="psum", bufs=8, space=bass.MemorySpace.PSUM) as psum,
        ):
            # Per-k-tile weight tiles + per-k first x chunk, interleaved, so
            # the first accumulation group starts after ~2 small DMAs instead
            # of two monolithic 4 MB loads (shrinks the kernel head).
            w_sb = {}   # wn -> list of [P, O_SHARD] tiles per k
            for wn in w_names:
                w_sb[wn] = [
                    wpool.tile([P, O_SHARD], dt16, tag=f"{wn}_{k}", name=f"w_sb_{wn}_{k}")
                    for k in range(K_TILES)
                ]
            x0_sb = {}  # xn -> list of [P, S_CHUNK] tiles per k (chunk 0)
            for xn in x_names:
                x0_sb[xn] = [
                    x0pool.tile([P, S_CHUNK], dt16, tag=f"{xn}0_{k}", name=f"x0_sb_{xn}_{k}")
                    for k in range(K_TILES)
                ]
            # w on the SP HWDGE ring, x on the ACT HWDGE ring -> the two
            # streams transfer concurrently and stay ahead of the k-outer
            # matmul order below.
            for k in range(K_TILES):
                for wn in w_names:
                    nc.sync.dma_start(
                        w_sb[wn][k][:],
                        handles[wn][k * P:(k + 1) * P, :],
                    )
                for xn in x_names:
                    nc.scalar.dma_start(
                        x0_sb[xn][k][:],
                        handles[xn][k * P:(k + 1) * P, 0:S_CHUNK],
                    )

            for c in range(N_CHUNKS):
                x_sb = {}
                if c == 0:
                    def x_tile(xn, k, ss):
                        return x0_sb[xn][k][:, ss * P:(ss + 1) * P]
                else:
                    for xn in x_names:
                        x_sb[xn] = xpool.tile(
                            [P, K_TILES, S_CHUNK], dt16, tag=xn, name=f"x_sb_{xn}"
                        )
                        nc.scalar.dma_start(
                            x_sb[xn][:],
                            handles[xn][:, c * S_CHUNK:(c + 1) * S_CHUNK].rearrange(
                                "(k p) s -> p k s", p=P
                            ),
                        )

                    def x_tile(xn, k, ss, x_sb=x_sb):
                        return x_sb[xn][:, k, ss * P:(ss + 1) * P]
                # k-outer, ss-inner: 4 PSUM accumulation groups run in
                # parallel, so k-tile k isn't needed until ~k*0.86us — the
                # streamed chunk-0 loads stay ahead of consumption.
                pts = [
                    psum.tile([P, O_SHARD], mybir.dt.float32, tag="pt", name=f"pt{ss}")
                    for ss in range(S_SUB)
                ]
                n_k = n_terms * K_TILES
                ki = 0
                for xn, wn in names:
                    for k in range(K_TILES):
                        for ss in range(S_SUB):
                            nc.tensor.matmul(
                                pts[ss][:],
                                x_tile(xn, k, ss),
                                w_sb[wn][k][:],
                                start=(ki == 0),
                                stop=(ki == n_k - 1),
                            )
                        ki += 1
                for ss in range(S_SUB):
                    o_sb = opool.tile([P, O_SHARD], mybir.dt.float32)
                    nc.vector.tensor_copy(o_sb[:], pts[ss][:])
                    s0 = c * S_CHUNK + ss * P
                    nc.sync.dma_start(out[s0:s0 + P, :], o_sb[:])
    nc.compile()
    return nc


def _install_ntff_hook():
    """Register the axon NTFF profiling hook if the image's antenv lacks it.

    Only used when PROFILE=True (test harness); grading never hits this.
    """
    import sys
    import types

    if "antenv.axon_hooks" in sys.modules:
        return
    try:
        from trn_agent_boot.trn_boot import _ntff_profile_via_ctypes
    except ImportError:
        return
    try:
        hook = _ntff_profile_via_ctypes("/opt/axon/libaxon_pjrt.so")
    except OSError:
        return
    m = types.ModuleType("antenv.axon_hooks")
    m.get_axon_ntff_profile_hook = lambda: hook
    m.set_axon_ntff_profile_hook = lambda h: None
    sys.modules["antenv.axon_hooks"] = m


def _get_nc():
    key = MODE
    if key not in _CACHE:
        if MODE == "fp16":
            _CACHE[key] = _build_fp16(split=False)
        elif MODE == "bf16":
            _CACHE[key] = _build_fp16(split=False, dt16=mybir.dt.bfloat16)
        elif MODE == "split":
            _CACHE[key] = _build_fp16(split=True)
        else:
            raise ValueError(f"unknown MODE {MODE}")
    return _CACHE[key]


def kernel(x: np.ndarray, weight: np.ndarray) -> np.ndarray:
    global LAST_PROFILE
    b, s, k = x.shape
    assert (b * s, k) == (S, K) and weight.shape == (O, K)

    xT = np.ascontiguousarray(x.reshape(S, K).T)          # [K, S] f32
    wT = np.ascontiguousarray(weight.T)                   # [K, O] f32

    if MODE in ("fp16", "bf16"):
        np16 = np.float16 if MODE == "fp16" else ml_dtypes.bfloat16
        xT16 = xT.astype(np16)
        wT16 = wT.astype(np16)
        in_maps = [
            {"x": xT16, "w": np.ascontiguousarray(wT16[:, c * O_SHARD:(c + 1) * O_SHARD])}
            for c in range(N_CORES)
        ]
    else:
        x_hi = xT.astype(np.float16)
        x_lo = (xT - x_hi.astype(np.float32)).astype(np.float16)
        w_hi = wT.astype(np.float16)
        w_lo = (wT - w_hi.astype(np.float32)).astype(np.float16)
        in_maps = [
            {
                "x_hi": x_hi,
                "x_lo": x_lo,
                "w_hi": np.ascontiguousarray(w_hi[:, c * O_SHARD:(c + 1) * O_SHARD]),
                "w_lo": np.ascontiguousarray(w_lo[:, c * O_SHARD:(c + 1) * O_SHARD]),
            }
            for c in range(N_CORES)
        ]

    if PROFILE:
        _install_ntff_hook()
    nc = _get_nc()
    res = run_bass_kernel_spmd(
        nc,
        in_maps,
        core_ids=list(range(N_CORES)),
        trace=PROFILE,
        trace_cores=[0] if PROFILE else None,
    )
    LAST_PROFILE = res

    full = np.empty((S, O), dtype=np.float32)
    for c in range(N_CORES):
        full[:, c * O_SHARD:(c + 1) * O_SHARD] = res.results[c]["out"]
    return full.reshape(b, s, O)



# revision 5
# speedup vs baseline: 1.0479x; 1.0479x over previous
"""Trainium2 Bass kernel for nn_LinearWithGroupedConv (out = x @ weight.T).

Full-input contract: kernel(x=[4,2048,4096] f32, weight=[4096,4096] f32)
-> [4,2048,4096] f32.

Strategy (tensor-parallel, column sharding per the hint):
  - out[s, o] = sum_k x[s, k] * weight[o, k];  S=8192 (4*2048), K=4096, O=4096.
  - Shard `weight` over out_feature across 8 cores (512 columns each),
    replicate x. Each core computes out_shard [8192, 512]; host concats.
  - fp16 operands (PSUM accumulation is fp32; rel err ~2e-4).
  - Host pre-lays-out x as [128p, chunk, ktile, s] and w as [128p, ktile, o]
    so every DMA moves large per-partition-contiguous lines (16-32 KB) --
    the previous [K, S] layout produced 1 KB descriptors that throttled the
    rings to ~half rate and starved the PE at the first chunk boundary
    (18 us stall + a HAM re-throttle to 1.2 GHz).
  - DMA plan: weights stream on the SP(sync) HWDGE ring in k-blocks
    [1,7,8,8,8] (first matmul only needs k=0), first-chunk x streams in the
    same block pattern on the ACT(scalar) ring, later chunks alternate
    rings (2 MB contiguous transfers). Output tiles go out via the gpsimd
    SWDGE queue so they never queue behind input streams.
  - A few tiny warm-up matmuls on a memset tile run during the DMA head so
    the PE HAM clock-gate reaches 8/8 (2.4 GHz) before the real matmuls.
  - Per chunk (256 s-cols): 2 PSUM accumulation groups x 32 k-tile matmuls
    ([128,128] stationary x [128,512] moving), DVE copy to SBUF, DMA out.
"""

import numpy as np

import concourse.bass as bass
import concourse.mybir as mybir
import concourse.tile as tile
from concourse import bacc
from concourse.bass_utils import run_bass_kernel_spmd

N_CORES = 8
S = 8192          # 4 * 2048 sequence rows
K = 4096          # in_feature (contraction)
O = 4096          # out_feature
O_SHARD = O // N_CORES          # 512
P = 128
K_TILES = K // P                # 32
S_CHUNK = 256                   # seq columns per streamed x chunk
S_SUB = S_CHUNK // P            # 2 psum groups per chunk
N_CHUNKS = S // S_CHUNK         # 32
K_BLOCKS = [(0, 1), (1, 8), (8, 16), (16, 24), (24, 32)]

MODE = "fp16"            # informational; single fp16 path
PROFILE = False          # test.py sets True to capture an NTFF trace
LAST_PROFILE = None      # BassKernelResults of the last run when PROFILE

_CACHE = {}


def _build(dt16=mybir.dt.float16):
    nc = bacc.Bacc(None, target_bir_lowering=False)

    x = nc.dram_tensor("x", [P, N_CHUNKS, K_TILES, S_CHUNK], dt16,
                       kind="ExternalInput")
    w = nc.dram_tensor("w", [P, K_TILES, O_SHARD], dt16, kind="ExternalInput")
    out = nc.dram_tensor("out", [S, O_SHARD], mybir.dt.float32,
                         kind="ExternalOutput")

    with tile.TileContext(nc) as tc:
        with (
            tc.tile_pool(name="wpool", bufs=1) as wpool,
            tc.tile_pool(name="x0pool", bufs=1) as x0pool,
            tc.tile_pool(name="xpool", bufs=3) as xpool,
            tc.tile_pool(name="opool", bufs=4) as opool,
            tc.tile_pool(name="warmsb", bufs=1) as warmsb,
            tc.tile_pool(name="psum", bufs=7, space=bass.MemorySpace.PSUM) as psum,
            tc.tile_pool(name="warmps", bufs=1, space=bass.MemorySpace.PSUM) as warmps,
        ):
            # PE warm-up during the DMA head: HAM un-throttles after ~3.4us
            # of sustained PE activity, so a burst of tiny matmuls here gets
            # the clock to 2.4 GHz sooner than the first real matmul would.
            warm_sb = warmsb.tile([P, P], dt16)
            nc.gpsimd.memset(warm_sb[:], 0.0)
            warm_ps = warmps.tile([P, 64], mybir.dt.float32)
            for _ in range(6):
                nc.tensor.matmul(warm_ps[:], warm_sb[:], warm_sb[:, 0:64],
                                 start=True, stop=True)

            # Weights on the SP ring / chunk-0 x on the ACT ring, k-blocked
            # and interleaved so the k=0 tiles of both land first and the
            # first matmul can start ~1.5us after the rings open.
            w_sb = []
            x0_sb = []
            for (k0, k1) in K_BLOCKS:
                wt = wpool.tile([P, k1 - k0, O_SHARD], dt16, name=f"w_{k0}")
                nc.sync.dma_start(wt[:], w[:, k0:k1, :])
                w_sb.append(wt)
                xt = x0pool.tile([P, k1 - k0, S_CHUNK], dt16, name=f"x0_{k0}")
                nc.scalar.dma_start(xt[:], x[:, 0, k0:k1, :])
                x0_sb.append(xt)

            # Chunks 1-2 also stream in ~0.5 MB k-blocks: during the head the
            # rings lag the PE slightly, and block granularity keeps each
            # individual stall well under the ~3.4us HAM re-throttle window.
            PREFIX = [1, 2]
            xp_sb = {}
            for c in PREFIX:
                eng = nc.scalar if (c % 2 == 1) else nc.sync
                blks = []
                for (k0, k1) in [(0, 8), (8, 16), (16, 24), (24, 32)]:
                    t = x0pool.tile([P, k1 - k0, S_CHUNK], dt16,
                                    name=f"xp{c}_{k0}")
                    eng.dma_start(t[:], x[:, c, k0:k1, :])
                    blks.append((k0, k1, t))
                xp_sb[c] = blks

            # Later chunks: single 2 MB fully-contiguous DMA each, alternating
            # rings.  bufs=3 keeps 2 chunks of prefetch in flight; the slot
            # wait throttles each ring to the consumption rate.
            xc_sb = {}
            for c in range(PREFIX[-1] + 1, N_CHUNKS):
                t = xpool.tile([P, K_TILES, S_CHUNK], dt16, tag="xc", name="xc")
                eng = nc.scalar if (c % 2 == 1) else nc.sync
                eng.dma_start(t[:], x[:, c, :, :])
                xc_sb[c] = t

            def x_ap(c, k, ss):
                if c == 0:
                    for bi, (k0, k1) in enumerate(K_BLOCKS):
                        if k0 <= k < k1:
                            return x0_sb[bi][:, k - k0, ss * P:(ss + 1) * P]
                if c in xp_sb:
                    for (k0, k1, t) in xp_sb[c]:
                        if k0 <= k < k1:
                            return t[:, k - k0, ss * P:(ss + 1) * P]
                return xc_sb[c][:, k, ss * P:(ss + 1) * P]

            def w_ap(k):
                for bi, (k0, k1) in enumerate(K_BLOCKS):
                    if k0 <= k < k1:
                        return w_sb[bi][:, k - k0, :]

            for c in range(N_CHUNKS):
                pts = [
                    psum.tile([P, O_SHARD], mybir.dt.float32, tag="pt",
                              name=f"pt{ss}")
                    for ss in range(S_SUB)
                ]
                for k in range(K_TILES):
                    for ss in range(S_SUB):
                        nc.tensor.matmul(
                            pts[ss][:],
                            x_ap(c, k, ss),
                            w_ap(k),
                            start=(k == 0),
                            stop=(k == K_TILES - 1),
                        )
                # Outputs ride the HWDGE ring opposite to this chunk's input
                # ring (SWDGE drains ~6us at kernel end; HWDGE doesn't).
                oeng = nc.sync if (c % 2 == 1) else nc.scalar
                for ss in range(S_SUB):
                    o_sb = opool.tile([P, O_SHARD], mybir.dt.float32)
                    nc.vector.tensor_copy(o_sb[:], pts[ss][:])
                    s0 = c * S_CHUNK + ss * P
                    oeng.dma_start(out[s0:s0 + P, :], o_sb[:])
    nc.compile()
    return nc


def _install_ntff_hook():
    """Register the axon NTFF profiling hook if the image's antenv lacks it.

    Only used when PROFILE=True (test harness); grading never hits this.
    """
    import sys
    import types

    if "antenv.axon_hooks" in sys.modules:
        return
    try:
        from trn_agent_boot.trn_boot import _ntff_profile_via_ctypes
    except ImportError:
        return
    try:
        hook = _ntff_profile_via_ctypes("/opt/axon/libaxon_pjrt.so")
    except OSError:
        return
    m = types.ModuleType("antenv.axon_hooks")
    m.get_axon_ntff_profile_hook = lambda: hook
    m.set_axon_ntff_profile_hook = lambda h: None
    sys.modules["antenv.axon_hooks"] = m


def _get_nc():
    if "nc" not in _CACHE:
        _CACHE["nc"] = _build()
    return _CACHE["nc"]


def kernel(x: np.ndarray, weight: np.ndarray) -> np.ndarray:
    global LAST_PROFILE
    b, s, k = x.shape
    assert (b * s, k) == (S, K) and weight.shape == (O, K)

    xT16 = np.ascontiguousarray(x.reshape(S, K).T).astype(np.float16)  # [K, S]
    # [ktile, p, chunk, s] -> [p, chunk, ktile, s]: every chunk slice is
    # per-partition contiguous (16 KB lines).
    x_arr = np.ascontiguousarray(
        xT16.reshape(K_TILES, P, N_CHUNKS, S_CHUNK).transpose(1, 2, 0, 3))
    wT16 = weight.T.astype(np.float16)                                 # [K, O]

    in_maps = []
    for c in range(N_CORES):
        w_c = np.ascontiguousarray(
            wT16[:, c * O_SHARD:(c + 1) * O_SHARD]
            .reshape(K_TILES, P, O_SHARD).transpose(1, 0, 2))
        in_maps.append({"x": x_arr, "w": w_c})

    if PROFILE:
        _install_ntff_hook()
    nc = _get_nc()
    res = run_bass_kernel_spmd(
        nc,
        in_maps,
        core_ids=list(range(N_CORES)),
        trace=PROFILE,
        trace_cores=[0] if PROFILE else None,
    )
    LAST_PROFILE = res

    full = np.empty((S, O), dtype=np.float32)
    for c in range(N_CORES):
        full[:, c * O_SHARD:(c + 1) * O_SHARD] = res.results[c]["out"]
    return full.reshape(b, s, O)
